# revision 18
# baseline (speedup 1.0000x reference)
"""TRN2 Bass kernel for nn_MultiHeadBatchedMixers (MoE token/channel mixer).

Strategy: expert-parallel with host-side routing. The (expert, head) work
units are bin-packed across 8 cores; all MoE gather/scatter happens on the
host, so the device runs a dense, static matmul chain per core:

  per (b,h,k) triple with expert e (two triples sharing (e,h) batched as a
  "pair", stacked along the partition dim):
    h1  = xT.T @ w1tT                   (token mix, contract n=256)
    h2T = h1-chunks.T @ blkdiag(w1cT)   (channel mix, transposed out)
    h2g = gelu_tanh(h2T + b1)
    o1  = h2g-chunks.T @ w2tT           (token mix back, contract hid=512)
    o2  = blkdiag(w2cT).T @ o1          (channel mix)
    out = ew * (o2 + b2)

All matmul operands are fp16 (fp32 PSUM accumulation); biases/outputs fp32.
Host pre-transposes every tensor so the device does zero transposes, and
DMAs are batched per (expert,head)-slot (>=0.5MB each) for near-peak HBM BW.
"""
import numpy as np

E, H, N, HD, HID = 8, 12, 256, 64, 512
B, TOPK = 32, 2
NCORES = 8
UNITS_PER_CORE = E * H // NCORES  # 12
WCOLS = 1024 + 128 + 1024 + 128   # packed weight tile columns (f16)
BCOLS = 512 + 256                 # packed bias tile columns (f32)

_last = {}  # introspection for test harness

# tuning knobs (pool buffer counts)
import os as _os
TUNE = dict(wpool=4, xpool=3, spool=5, opool=3, ps1=2, ps2=2, ps3=2, ps4=2,
            out16=bool(int(_os.environ.get("KERNEL_OUT16", "0"))),
            # xcfirst: apply channel-mix to x first (associativity) — halves
            # the layer-1 PSUM evacuation and shrinks PE column count
            xcfirst=bool(int(_os.environ.get("KERNEL_XCFIRST", "1"))))


def _route(expert_indices, expert_weights):
    """Host-side routing: unit = (e,h). Returns per-core slot structure."""
    idx = np.asarray(expert_indices).astype(np.int64)
    ew = np.asarray(expert_weights).astype(np.float64)
    units = {(e, h): [] for e in range(E) for h in range(H)}
    for b in range(B):
        for h in range(H):
            if idx[b, h, 0] == idx[b, h, 1]:
                units[(int(idx[b, h, 0]), h)].append(
                    (b, float(ew[b, h, 0] + ew[b, h, 1])))
            else:
                for k in range(TOPK):
                    units[(int(idx[b, h, k]), h)].append((b, float(ew[b, h, k])))

    ulist = []
    for (e, h), triples in units.items():
        npairs = (len(triples) + 1) // 2
        ulist.append([npairs, e, h, triples])
    # LPT bin-packing into NCORES bins of exactly UNITS_PER_CORE units
    ulist.sort(key=lambda t: -t[0])
    bins = [[] for _ in range(NCORES)]
    loads = [0] * NCORES
    for u in ulist:
        cands = [c for c in range(NCORES) if len(bins[c]) < UNITS_PER_CORE]
        c = min(cands, key=lambda c: loads[c])
        bins[c].append(u)
        loads[c] += u[0]
    for c in range(NCORES):
        bins[c].sort(key=lambda t: -t[0])
    caps = [max(bins[c][s][0] for c in range(NCORES)) for s in range(UNITS_PER_CORE)]
    return bins, caps


def _pack(x, bins, caps, fc1_tok, fc1_chan, fc1_bias, fc2_tok, fc2_chan, fc2_bias,
          dt16np):
    """Build per-core dense input arrays (all pre-transposed layouts)."""
    pairs_total = sum(caps)
    P = max(pairs_total, 1)
    x = np.asarray(x, np.float32)
    f32 = np.float32

    xp = np.zeros((NCORES, 128, P * 256), dt16np)
    wcat = np.zeros((NCORES, UNITS_PER_CORE, 128, WCOLS), dt16np)
    bcat = np.zeros((NCORES, UNITS_PER_CORE, 128, BCOLS), f32)
    ewp = np.zeros((NCORES, 128, P), f32)
    meta = [[] for _ in range(NCORES)]  # per core: pair slot -> [(t, b, h)]

    f1t = np.asarray(fc1_tok, f32)
    f1c = np.asarray(fc1_chan, f32)
    f1b = np.asarray(fc1_bias, f32)
    f2t = np.asarray(fc2_tok, f32)
    f2c = np.asarray(fc2_chan, f32)
    f2b = np.asarray(fc2_bias, f32)

    for c in range(NCORES):
        p = 0
        for s, (npairs_u, e, h, triples) in enumerate(bins[c]):
            # w1tT: [n, nc*512+f] = fc1_tok[e,h,f,nc*128+n]
            a = f1t[e, h].T.reshape(2, 128, 512).transpose(1, 0, 2).reshape(128, 1024)
            wcat[c, s, :, 0:1024] = a.astype(dt16np)
            # blkdiag(w1cT)
            wc = f1c[e, h].T.astype(dt16np)
            wcat[c, s, :64, 1024:1088] = wc
            wcat[c, s, 64:, 1088:1152] = wc
            # w2tT: [f_in_chunk, j*256+n] = fc2_tok[e,h,n,j*128+f]
            a = f2t[e, h].T.reshape(4, 128, 256).transpose(1, 0, 2).reshape(128, 1024)
            wcat[c, s, :, 1152:2176] = a.astype(dt16np)
            wc = f2c[e, h].T.astype(dt16np)
            wcat[c, s, :64, 2176:2240] = wc
            wcat[c, s, 64:, 2240:2304] = wc
            # b1 tile: [f, j*128 + t*64 + ch] = fc1_bias[e,h,ch,j*128+f]
            a = f1b[e, h].reshape(64, 4, 128).transpose(2, 1, 0)  # (f128, j, ch)
            a = np.stack([a, a], axis=2)  # (f128, j, t, ch)
            bcat[c, s, :, 0:512] = a.reshape(128, 512)
            # b2 tile: [t*64+ch, n]
            bcat[c, s, :, 512:768] = np.concatenate([f2b[e, h], f2b[e, h]], axis=0)

            for ip in range(caps[s]):
                t0 = triples[2 * ip] if 2 * ip < len(triples) else None
                t1 = triples[2 * ip + 1] if 2 * ip + 1 < len(triples) else None
                pair_triples = []
                for t, tr in enumerate((t0, t1)):
                    if tr is None:
                        continue
                    b_, wgt = tr
                    if TUNE.get("xcfirst"):
                        # x natural (d,n), triples stacked on partitions
                        xp[c, t * 64:(t + 1) * 64, p * 256:(p + 1) * 256] = \
                            x[b_, h].astype(dt16np)
                    else:
                        xT = x[b_, h].T.reshape(2, 128, 64)  # (nc, n, d)
                        for ncn in range(2):
                            col = p * 256 + ncn * 128 + t * 64
                            xp[c, :, col:col + 64] = xT[ncn].astype(dt16np)
                    ewp[c, t * 64:(t + 1) * 64, p] = wgt
                    pair_triples.append((t, b_, h))
                meta[c].append(pair_triples)
                p += 1
        assert p == pairs_total
    return dict(xp=xp, wcat=wcat, bcat=bcat, ewp=ewp, meta=meta,
                pairs_total=pairs_total)


def _build_nc(caps, pairs_total, with_bias, dt16, reps=1, sim_act=False):
    import concourse.mybir as mybir
    import concourse.tile as tile
    from concourse import bacc

    F32 = mybir.dt.float32
    AF = mybir.ActivationFunctionType
    GELU = AF.Tanh if sim_act else AF.Gelu_apprx_tanh

    nc = bacc.Bacc("TRN2", target_bir_lowering=False, debug=False,
                   num_devices=NCORES)
    U = UNITS_PER_CORE
    P = max(pairs_total, 1)
    xp_d = nc.declare_dram_parameter("xp", [128, P * 256], dt16, False)
    w_d = nc.declare_dram_parameter("wcat", [U, 128, WCOLS], dt16, False)
    if with_bias:
        b_d = nc.declare_dram_parameter("bcat", [U, 128, BCOLS], F32, False)
    ew_d = nc.declare_dram_parameter("ewp", [128, P], F32, False)
    ODT = dt16 if TUNE.get("out16") else F32
    out_d = nc.declare_dram_parameter("out", [128, P * 256], ODT, True)

    maxcap = max(caps) if caps else 1

    with tile.TileContext(nc) as tc:
        with (
            tc.tile_pool(name="wpool", bufs=TUNE["wpool"]) as wpool,
            tc.tile_pool(name="xpool", bufs=TUNE["xpool"]) as xpool,
            tc.tile_pool(name="spool", bufs=TUNE["spool"]) as spool,
            tc.tile_pool(name="opool", bufs=TUNE["opool"]) as opool,
            tc.tile_pool(name="cpool", bufs=1) as cpool,
            tc.tile_pool(name="ps1", bufs=TUNE["ps1"], space="PSUM") as ps1,
            tc.tile_pool(name="ps2", bufs=TUNE["ps2"], space="PSUM") as ps2,
            tc.tile_pool(name="ps3", bufs=TUNE["ps3"], space="PSUM") as ps3,
            tc.tile_pool(name="ps4", bufs=TUNE["ps4"], space="PSUM") as ps4,
        ):
            ewt = cpool.tile([128, P], F32)
            nc.sync.dma_start(ewt[:, :], ew_d[:, :])

            # software-pipelined emission: PE stream per step i is
            #   M1(i), M2(i-1), M4(i-2), M5(i-3)
            # so PE never waits on the DVE/ACT stage of the same pair.
            xcfirst = TUNE.get("xcfirst")

            def stage_a(ctx):  # layer-1 first matmul + PSUM evacuation
                xt = ctx["xs"][:, ctx["ip"] * 256:(ctx["ip"] + 1) * 256]
                wt = ctx["wt"]
                if xcfirst:
                    # xcT[n, (t,c)] = blkdiag(w1cT).T applied to natural-layout
                    # x (channel mix first; associativity with the token mix)
                    xcp = ps1.tile([128, 256], F32, tag="h1p")
                    for ncn in range(2):
                        nc.tensor.matmul(xcp[:, ncn * 128:(ncn + 1) * 128],
                                         xt[:, ncn * 128:(ncn + 1) * 128],
                                         wt[:, 1024:1152], start=True, stop=True)
                    h1s = spool.tile([128, 256], dt16, tag="h1s")
                else:
                    h1p = ps1.tile([128, 512], F32, tag="h1p")
                    nc.tensor.matmul(h1p[:, :], xt[:, 0:128], wt[:, 0:512],
                                     start=True, stop=False)
                    nc.tensor.matmul(h1p[:, :], xt[:, 128:256], wt[:, 512:1024],
                                     start=False, stop=True)
                    xcp = h1p
                    h1s = spool.tile([128, 512], dt16, tag="h1s")
                nc.vector.tensor_copy(h1s[:, :], xcp[:, :])
                ctx["h1s"] = h1s

            def stage_b(ctx):  # layer-1 second matmul + bias + gelu
                wt, h1s = ctx["wt"], ctx["h1s"]
                h2p = ps2.tile([128, 512], F32, tag="h2p")
                if xcfirst:
                    # h2T[f, (t,c)] = w1tT-blocks.T @ xcT, accumulated over n
                    for j in range(4):
                        for ncn in range(2):
                            nc.tensor.matmul(
                                h2p[:, j * 128:(j + 1) * 128],
                                wt[:, ncn * 512 + j * 128:ncn * 512 + (j + 1) * 128],
                                h1s[:, ncn * 128:(ncn + 1) * 128],
                                start=(ncn == 0), stop=(ncn == 1))
                else:
                    for j in range(4):
                        nc.tensor.matmul(h2p[:, j * 128:(j + 1) * 128],
                                         h1s[:, j * 128:(j + 1) * 128],
                                         wt[:, 1024:1152], start=True, stop=True)
                h2s = spool.tile([128, 512], dt16, tag="h2s")
                if with_bias:
                    h2a = spool.tile([128, 512], F32, tag="h2a")
                    nc.vector.tensor_add(h2a[:, :], h2p[:, :], ctx["bt"][:, 0:512])
                    nc.scalar.activation(h2s[:, :], h2a[:, :], GELU)
                else:
                    nc.scalar.activation(h2s[:, :], h2p[:, :], GELU)
                ctx["h2s"] = h2s

            def stage_c(ctx):  # M4 + o1 evacuation
                wt, h2s = ctx["wt"], ctx["h2s"]
                o1p = ps3.tile([128, 256], F32, tag="o1p")
                for j in range(4):
                    nc.tensor.matmul(o1p[:, :], h2s[:, j * 128:(j + 1) * 128],
                                     wt[:, 1152 + j * 256:1152 + (j + 1) * 256],
                                     start=(j == 0), stop=(j == 3))
                o1s = spool.tile([128, 256], dt16, tag="o1s")
                nc.vector.tensor_copy(o1s[:, :], o1p[:, :])
                ctx["o1s"] = o1s

            def stage_d(ctx):  # M5 + scale (+bias) into slot output tile
                wt, o1s, p = ctx["wt"], ctx["o1s"], ctx["p"]
                o2p = ps4.tile([128, 256], F32, tag="o2p")
                nc.tensor.matmul(o2p[:, :], wt[:, 2176:2304], o1s[:, :],
                                 start=True, stop=True)
                ot = ctx["os"][:, ctx["ip"] * 256:(ctx["ip"] + 1) * 256]
                if with_bias:
                    o2a = opool.tile([128, 256], F32, tag="o2a")
                    nc.vector.tensor_add(o2a[:, :], o2p[:, :], ctx["bt"][:, 512:768])
                    nc.scalar.activation(ot, o2a[:, :], AF.Copy, scale=ewt[:, p:p + 1])
                else:
                    nc.scalar.activation(ot, o2p[:, :], AF.Copy, scale=ewt[:, p:p + 1])
                if ctx["last_in_slot"]:
                    nc.sync.dma_start(
                        out_d[:, ctx["slot_p0"] * 256:(ctx["p"] + 1) * 256],
                        ctx["os"][:, :ctx["cap"] * 256])

            for _rep in range(reps):
                # flat pair list with slot context
                flat = []
                p = 0
                for s in range(UNITS_PER_CORE):
                    cap = caps[s]
                    if cap == 0:
                        continue
                    for ip in range(cap):
                        flat.append(dict(s=s, ip=ip, p=p, cap=cap,
                                         slot_p0=p - ip,
                                         last_in_slot=(ip == cap - 1)))
                        p += 1
                # run pipeline with slot-DMA prefetch PRE pairs ahead
                slot_tiles = {}
                LAG = TUNE.get("lag", 3)
                PRE = TUNE.get("prefetch", 3)

                def prefetch_slot(ctx):
                    s = ctx["s"]
                    if s in slot_tiles:
                        return
                    wt = wpool.tile([128, WCOLS], dt16, tag="wt")
                    nc.sync.dma_start(wt[:, :], w_d[s])
                    bt = None
                    if with_bias:
                        bt = wpool.tile([128, BCOLS], F32, tag="bt")
                        nc.sync.dma_start(bt[:, :], b_d[s])
                    xs = xpool.tile([128, maxcap * 256], dt16, tag="xs")
                    nc.sync.dma_start(
                        xs[:, :ctx["cap"] * 256],
                        xp_d[:, ctx["slot_p0"] * 256:(ctx["slot_p0"] + ctx["cap"]) * 256])
                    os_ = opool.tile([128, maxcap * 256], ODT, tag="os")
                    slot_tiles[s] = (wt, bt, xs, os_)

                for i in range(len(flat) + LAG):
                    if i < len(flat):
                        ctx = flat[i]
                        if i == 0:
                            for j in range(min(PRE + 1, len(flat))):
                                prefetch_slot(flat[j])
                        if i + PRE < len(flat):
                            prefetch_slot(flat[i + PRE])
                        ctx["wt"], ctx["bt"], ctx["xs"], ctx["os"] = slot_tiles[ctx["s"]]
                        stage_a(ctx)
                    if i - 1 >= 0 and i - 1 < len(flat):
                        stage_b(flat[i - 1])
                    if i - 2 >= 0 and i - 2 < len(flat):
                        stage_c(flat[i - 2])
                    if i - 3 >= 0 and i - 3 < len(flat):
                        stage_d(flat[i - 3])
    nc.compile()
    return nc


def kernel(x, expert_indices, expert_weights, fc1_tok, fc1_chan, fc1_bias,
           fc2_tok, fc2_chan, fc2_bias):
    import concourse.mybir as mybir
    from concourse.bass_utils import run_bass_kernel_spmd

    dt16 = mybir.dt.float16
    dt16np = np.float16

    bins, caps = _route(expert_indices, expert_weights)
    with_bias = bool(np.any(np.asarray(fc1_bias)) or np.any(np.asarray(fc2_bias)))
    packed = _pack(x, bins, caps, fc1_tok, fc1_chan, fc1_bias, fc2_tok, fc2_chan,
                   fc2_bias, dt16np)
    pairs_total = packed["pairs_total"]

    nc = _build_nc(caps, pairs_total, with_bias, dt16)

    in_names = ["xp", "wcat", "ewp"] + (["bcat"] if with_bias else [])
    in_maps = [{k: packed[k][c] for k in in_names} for c in range(NCORES)]
    core_ids = list(range(NCORES))

    res = run_bass_kernel_spmd(nc, in_maps, core_ids)

    out = np.zeros((B, H, HD, N), np.float32)
    for c in range(NCORES):
        oarr = res.results[c]["out"]  # [128, P*256]
        for p, pair_triples in enumerate(packed["meta"][c]):
            for (t, b_, h) in pair_triples:
                out[b_, h] += oarr[t * 64:(t + 1) * 64, p * 256:(p + 1) * 256]

    _last.clear()
    _last.update(nc=nc, in_maps=in_maps, res=res, packed=packed, caps=caps,
                 with_bias=with_bias, bins=bins)
    return out


# revision 21
# speedup vs baseline: 1.1757x; 1.1757x over previous
"""TRN2 Bass kernel for nn_MultiHeadBatchedMixers (MoE token/channel mixer).

Strategy: expert-parallel with host-side routing. The (expert, head) work
units are bin-packed across 8 cores; all MoE gather/scatter happens on the
host, so the device runs a dense, static matmul chain per core:

  per (b,h,k) triple with expert e (two triples sharing (e,h) batched as a
  "pair", stacked along the partition dim):
    h1  = xT.T @ w1tT                   (token mix, contract n=256)
    h2T = h1-chunks.T @ blkdiag(w1cT)   (channel mix, transposed out)
    h2g = gelu_tanh(h2T + b1)
    o1  = h2g-chunks.T @ w2tT           (token mix back, contract hid=512)
    o2  = blkdiag(w2cT).T @ o1          (channel mix)
    out = ew * (o2 + b2)

All matmul operands are fp16 (fp32 PSUM accumulation); biases/outputs fp32.
Host pre-transposes every tensor so the device does zero transposes, and
DMAs are batched per (expert,head)-slot (>=0.5MB each) for near-peak HBM BW.
"""
import numpy as np

E, H, N, HD, HID = 8, 12, 256, 64, 512
B, TOPK = 32, 2
NCORES = 8
UNITS_PER_CORE = E * H // NCORES  # 12
WCOLS = 1024 + 128 + 1024 + 128   # packed weight tile columns (f16)
BCOLS = 512 + 256                 # packed bias tile columns (f32)

_last = {}  # introspection for test harness

# tuning knobs (pool buffer counts)
import os as _os
TUNE = dict(wpool=4, xpool=3, spool=5, opool=3, ps1=2, ps2=2, ps3=2, ps4=2,
            out16=bool(int(_os.environ.get("KERNEL_OUT16", "0"))),
            # xcfirst: apply channel-mix to x first (associativity) — halves
            # the layer-1 PSUM evacuation and shrinks PE column count
            xcfirst=bool(int(_os.environ.get("KERNEL_XCFIRST", "1"))),
            # splito2: split the o2 evacuation between DVE and ACT so both
            # evacuation engines carry 640 cols/pair instead of 512/768
            splito2=bool(int(_os.environ.get("KERNEL_SPLITO2", "0"))))


def _route(expert_indices, expert_weights):
    """Host-side routing: unit = (e,h). Returns per-core slot structure."""
    idx = np.asarray(expert_indices).astype(np.int64)
    ew = np.asarray(expert_weights).astype(np.float64)
    units = {(e, h): [] for e in range(E) for h in range(H)}
    for b in range(B):
        for h in range(H):
            if idx[b, h, 0] == idx[b, h, 1]:
                units[(int(idx[b, h, 0]), h)].append(
                    (b, float(ew[b, h, 0] + ew[b, h, 1])))
            else:
                for k in range(TOPK):
                    units[(int(idx[b, h, k]), h)].append((b, float(ew[b, h, k])))

    ulist = []
    for (e, h), triples in units.items():
        npairs = (len(triples) + 1) // 2
        ulist.append([npairs, e, h, triples])
    # LPT bin-packing into NCORES bins of exactly UNITS_PER_CORE units
    ulist.sort(key=lambda t: -t[0])
    bins = [[] for _ in range(NCORES)]
    loads = [0] * NCORES
    for u in ulist:
        cands = [c for c in range(NCORES) if len(bins[c]) < UNITS_PER_CORE]
        c = min(cands, key=lambda c: loads[c])
        bins[c].append(u)
        loads[c] += u[0]
    for c in range(NCORES):
        bins[c].sort(key=lambda t: -t[0])
    caps = [max(bins[c][s][0] for c in range(NCORES)) for s in range(UNITS_PER_CORE)]
    return bins, caps


def _pack(x, bins, caps, fc1_tok, fc1_chan, fc1_bias, fc2_tok, fc2_chan, fc2_bias,
          dt16np):
    """Build per-core dense input arrays (all pre-transposed layouts)."""
    pairs_total = sum(caps)
    P = max(pairs_total, 1)
    x = np.asarray(x, np.float32)
    f32 = np.float32

    xp = np.zeros((NCORES, 128, P * 256), dt16np)
    wcat = np.zeros((NCORES, UNITS_PER_CORE, 128, WCOLS), dt16np)
    bcat = np.zeros((NCORES, UNITS_PER_CORE, 128, BCOLS), f32)
    ewp = np.zeros((NCORES, 128, P), f32)
    meta = [[] for _ in range(NCORES)]  # per core: pair slot -> [(t, b, h)]

    f1t = np.asarray(fc1_tok, f32)
    f1c = np.asarray(fc1_chan, f32)
    f1b = np.asarray(fc1_bias, f32)
    f2t = np.asarray(fc2_tok, f32)
    f2c = np.asarray(fc2_chan, f32)
    f2b = np.asarray(fc2_bias, f32)

    for c in range(NCORES):
        p = 0
        for s, (npairs_u, e, h, triples) in enumerate(bins[c]):
            # w1tT: [n, nc*512+f] = fc1_tok[e,h,f,nc*128+n]
            a = f1t[e, h].T.reshape(2, 128, 512).transpose(1, 0, 2).reshape(128, 1024)
            wcat[c, s, :, 0:1024] = a.astype(dt16np)
            # blkdiag(w1cT)
            wc = f1c[e, h].T.astype(dt16np)
            wcat[c, s, :64, 1024:1088] = wc
            wcat[c, s, 64:, 1088:1152] = wc
            # w2tT: [f_in_chunk, j*256+n] = fc2_tok[e,h,n,j*128+f]
            a = f2t[e, h].T.reshape(4, 128, 256).transpose(1, 0, 2).reshape(128, 1024)
            wcat[c, s, :, 1152:2176] = a.astype(dt16np)
            wc = f2c[e, h].T.astype(dt16np)
            wcat[c, s, :64, 2176:2240] = wc
            wcat[c, s, 64:, 2240:2304] = wc
            # b1 tile: [f, j*128 + t*64 + ch] = fc1_bias[e,h,ch,j*128+f]
            a = f1b[e, h].reshape(64, 4, 128).transpose(2, 1, 0)  # (f128, j, ch)
            a = np.stack([a, a], axis=2)  # (f128, j, t, ch)
            bcat[c, s, :, 0:512] = a.reshape(128, 512)
            # b2 tile: [t*64+ch, n]
            bcat[c, s, :, 512:768] = np.concatenate([f2b[e, h], f2b[e, h]], axis=0)

            for ip in range(caps[s]):
                t0 = triples[2 * ip] if 2 * ip < len(triples) else None
                t1 = triples[2 * ip + 1] if 2 * ip + 1 < len(triples) else None
                pair_triples = []
                for t, tr in enumerate((t0, t1)):
                    if tr is None:
                        continue
                    b_, wgt = tr
                    if TUNE.get("xcfirst"):
                        # x natural (d,n), triples stacked on partitions
                        xp[c, t * 64:(t + 1) * 64, p * 256:(p + 1) * 256] = \
                            x[b_, h].astype(dt16np)
                    else:
                        xT = x[b_, h].T.reshape(2, 128, 64)  # (nc, n, d)
                        for ncn in range(2):
                            col = p * 256 + ncn * 128 + t * 64
                            xp[c, :, col:col + 64] = xT[ncn].astype(dt16np)
                    ewp[c, t * 64:(t + 1) * 64, p] = wgt
                    pair_triples.append((t, b_, h))
                meta[c].append(pair_triples)
                p += 1
        assert p == pairs_total
    return dict(xp=xp, wcat=wcat, bcat=bcat, ewp=ewp, meta=meta,
                pairs_total=pairs_total)


def _build_nc(caps, pairs_total, with_bias, dt16, reps=1, sim_act=False):
    import concourse.mybir as mybir
    import concourse.tile as tile
    from concourse import bacc

    F32 = mybir.dt.float32
    AF = mybir.ActivationFunctionType
    GELU = AF.Tanh if sim_act else AF.Gelu_apprx_tanh

    nc = bacc.Bacc("TRN2", target_bir_lowering=False, debug=False,
                   num_devices=NCORES)
    U = UNITS_PER_CORE
    P = max(pairs_total, 1)
    xp_d = nc.declare_dram_parameter("xp", [128, P * 256], dt16, False)
    w_d = nc.declare_dram_parameter("wcat", [U, 128, WCOLS], dt16, False)
    if with_bias:
        b_d = nc.declare_dram_parameter("bcat", [U, 128, BCOLS], F32, False)
    ew_d = nc.declare_dram_parameter("ewp", [128, P], F32, False)
    ODT = dt16 if TUNE.get("out16") else F32
    out_d = nc.declare_dram_parameter("out", [128, P * 256], ODT, True)

    maxcap = max(caps) if caps else 1

    with tile.TileContext(nc) as tc:
        with (
            tc.tile_pool(name="wpool", bufs=TUNE["wpool"]) as wpool,
            tc.tile_pool(name="xpool", bufs=TUNE["xpool"]) as xpool,
            tc.tile_pool(name="spool", bufs=TUNE["spool"]) as spool,
            tc.tile_pool(name="opool", bufs=TUNE["opool"]) as opool,
            tc.tile_pool(name="cpool", bufs=1) as cpool,
            tc.tile_pool(name="ps1", bufs=TUNE["ps1"], space="PSUM") as ps1,
            tc.tile_pool(name="ps2", bufs=TUNE["ps2"], space="PSUM") as ps2,
            tc.tile_pool(name="ps3", bufs=TUNE["ps3"], space="PSUM") as ps3,
            tc.tile_pool(name="ps4", bufs=TUNE["ps4"], space="PSUM") as ps4,
        ):
            ewt = cpool.tile([128, P], F32)
            nc.sync.dma_start(ewt[:, :], ew_d[:, :])

            # software-pipelined emission: PE stream per step i is
            #   M1(i), M2(i-1), M4(i-2), M5(i-3)
            # so PE never waits on the DVE/ACT stage of the same pair.
            xcfirst = TUNE.get("xcfirst")

            def stage_a(ctx):  # layer-1 first matmul + PSUM evacuation
                xt = ctx["xs"][:, ctx["ip"] * 256:(ctx["ip"] + 1) * 256]
                wt = ctx["wt"]
                if xcfirst:
                    # xcT[n, (t,c)] = blkdiag(w1cT).T applied to natural-layout
                    # x (channel mix first; associativity with the token mix)
                    xcp = ps1.tile([128, 256], F32, tag="h1p")
                    for ncn in range(2):
                        nc.tensor.matmul(xcp[:, ncn * 128:(ncn + 1) * 128],
                                         xt[:, ncn * 128:(ncn + 1) * 128],
                                         wt[:, 1024:1152], start=True, stop=True)
                    h1s = spool.tile([128, 256], dt16, tag="h1s")
                else:
                    h1p = ps1.tile([128, 512], F32, tag="h1p")
                    nc.tensor.matmul(h1p[:, :], xt[:, 0:128], wt[:, 0:512],
                                     start=True, stop=False)
                    nc.tensor.matmul(h1p[:, :], xt[:, 128:256], wt[:, 512:1024],
                                     start=False, stop=True)
                    xcp = h1p
                    h1s = spool.tile([128, 512], dt16, tag="h1s")
                nc.vector.tensor_copy(h1s[:, :], xcp[:, :])
                ctx["h1s"] = h1s

            def stage_b(ctx):  # layer-1 second matmul + bias + gelu
                wt, h1s = ctx["wt"], ctx["h1s"]
                h2p = ps2.tile([128, 512], F32, tag="h2p")
                if xcfirst:
                    # h2T[f, (t,c)] = w1tT-blocks.T @ xcT, accumulated over n
                    for j in range(4):
                        for ncn in range(2):
                            nc.tensor.matmul(
                                h2p[:, j * 128:(j + 1) * 128],
                                wt[:, ncn * 512 + j * 128:ncn * 512 + (j + 1) * 128],
                                h1s[:, ncn * 128:(ncn + 1) * 128],
                                start=(ncn == 0), stop=(ncn == 1))
                else:
                    for j in range(4):
                        nc.tensor.matmul(h2p[:, j * 128:(j + 1) * 128],
                                         h1s[:, j * 128:(j + 1) * 128],
                                         wt[:, 1024:1152], start=True, stop=True)
                h2s = spool.tile([128, 512], dt16, tag="h2s")
                if with_bias:
                    h2a = spool.tile([128, 512], F32, tag="h2a")
                    nc.vector.tensor_add(h2a[:, :], h2p[:, :], ctx["bt"][:, 0:512])
                    nc.scalar.activation(h2s[:, :], h2a[:, :], GELU)
                else:
                    nc.scalar.activation(h2s[:, :], h2p[:, :], GELU)
                ctx["h2s"] = h2s

            def stage_c(ctx):  # M4 + o1 evacuation
                wt, h2s = ctx["wt"], ctx["h2s"]
                o1p = ps3.tile([128, 256], F32, tag="o1p")
                for j in range(4):
                    nc.tensor.matmul(o1p[:, :], h2s[:, j * 128:(j + 1) * 128],
                                     wt[:, 1152 + j * 256:1152 + (j + 1) * 256],
                                     start=(j == 0), stop=(j == 3))
                o1s = spool.tile([128, 256], dt16, tag="o1s")
                nc.vector.tensor_copy(o1s[:, :], o1p[:, :])
                ctx["o1s"] = o1s

            def stage_d(ctx):  # M5 + scale (+bias) into slot output tile
                wt, o1s, p = ctx["wt"], ctx["o1s"], ctx["p"]
                o2p = ps4.tile([128, 256], F32, tag="o2p")
                nc.tensor.matmul(o2p[:, :], wt[:, 2176:2304], o1s[:, :],
                                 start=True, stop=True)
                ot = ctx["os"][:, ctx["ip"] * 256:(ctx["ip"] + 1) * 256]
                if with_bias:
                    o2a = opool.tile([128, 256], F32, tag="o2a")
                    nc.vector.tensor_add(o2a[:, :], o2p[:, :], ctx["bt"][:, 512:768])
                    nc.scalar.activation(ot, o2a[:, :], AF.Copy, scale=ewt[:, p:p + 1])
                elif TUNE.get("splito2"):
                    nc.vector.tensor_scalar_mul(ot[:, 0:128], o2p[:, 0:128],
                                                ewt[:, p:p + 1])
                    nc.scalar.activation(ot[:, 128:256], o2p[:, 128:256], AF.Copy,
                                         scale=ewt[:, p:p + 1])
                else:
                    nc.scalar.activation(ot, o2p[:, :], AF.Copy, scale=ewt[:, p:p + 1])
                if ctx["last_in_slot"]:
                    nc.sync.dma_start(
                        out_d[:, ctx["slot_p0"] * 256:(ctx["p"] + 1) * 256],
                        ctx["os"][:, :ctx["cap"] * 256])

            for _rep in range(reps):
                # flat pair list with slot context
                flat = []
                p = 0
                for s in range(UNITS_PER_CORE):
                    cap = caps[s]
                    if cap == 0:
                        continue
                    for ip in range(cap):
                        flat.append(dict(s=s, ip=ip, p=p, cap=cap,
                                         slot_p0=p - ip,
                                         last_in_slot=(ip == cap - 1)))
                        p += 1
                # run pipeline with slot-DMA prefetch PRE pairs ahead
                slot_tiles = {}
                LAG = TUNE.get("lag", 3)
                PRE = TUNE.get("prefetch", 3)

                def prefetch_slot(ctx):
                    s = ctx["s"]
                    if s in slot_tiles:
                        return
                    wt = wpool.tile([128, WCOLS], dt16, tag="wt")
                    nc.sync.dma_start(wt[:, :], w_d[s])
                    bt = None
                    if with_bias:
                        bt = wpool.tile([128, BCOLS], F32, tag="bt")
                        nc.sync.dma_start(bt[:, :], b_d[s])
                    xs = xpool.tile([128, maxcap * 256], dt16, tag="xs")
                    nc.sync.dma_start(
                        xs[:, :ctx["cap"] * 256],
                        xp_d[:, ctx["slot_p0"] * 256:(ctx["slot_p0"] + ctx["cap"]) * 256])
                    os_ = opool.tile([128, maxcap * 256], ODT, tag="os")
                    slot_tiles[s] = (wt, bt, xs, os_)

                for i in range(len(flat) + LAG):
                    if i < len(flat):
                        ctx = flat[i]
                        if i == 0:
                            for j in range(min(PRE + 1, len(flat))):
                                prefetch_slot(flat[j])
                        if i + PRE < len(flat):
                            prefetch_slot(flat[i + PRE])
                        ctx["wt"], ctx["bt"], ctx["xs"], ctx["os"] = slot_tiles[ctx["s"]]
                        stage_a(ctx)
                    if i - 1 >= 0 and i - 1 < len(flat):
                        stage_b(flat[i - 1])
                    if i - 2 >= 0 and i - 2 < len(flat):
                        stage_c(flat[i - 2])
                    if i - 3 >= 0 and i - 3 < len(flat):
                        stage_d(flat[i - 3])
    nc.compile()
    return nc


def kernel(x, expert_indices, expert_weights, fc1_tok, fc1_chan, fc1_bias,
           fc2_tok, fc2_chan, fc2_bias):
    import concourse.mybir as mybir
    from concourse.bass_utils import run_bass_kernel_spmd

    dt16 = mybir.dt.float16
    dt16np = np.float16

    bins, caps = _route(expert_indices, expert_weights)
    with_bias = bool(np.any(np.asarray(fc1_bias)) or np.any(np.asarray(fc2_bias)))
    packed = _pack(x, bins, caps, fc1_tok, fc1_chan, fc1_bias, fc2_tok, fc2_chan,
                   fc2_bias, dt16np)
    pairs_total = packed["pairs_total"]

    nc = _build_nc(caps, pairs_total, with_bias, dt16)

    in_names = ["xp", "wcat", "ewp"] + (["bcat"] if with_bias else [])
    in_maps = [{k: packed[k][c] for k in in_names} for c in range(NCORES)]
    core_ids = list(range(NCORES))

    res = run_bass_kernel_spmd(nc, in_maps, core_ids)

    out = np.zeros((B, H, HD, N), np.float32)
    for c in range(NCORES):
        oarr = res.results[c]["out"]  # [128, P*256]
        for p, pair_triples in enumerate(packed["meta"][c]):
            for (t, b_, h) in pair_triples:
                out[b_, h] += oarr[t * 64:(t + 1) * 64, p * 256:(p + 1) * 256]

    _last.clear()
    _last.update(nc=nc, in_maps=in_maps, res=res, packed=packed, caps=caps,
                 with_bias=with_bias, bins=bins)
    return out
